# revision 4
# baseline (speedup 1.0000x reference)
"""Trainium2 Bass kernel for nn_MultiHeadAttention_26259430048704.

Multi-head attention with additive bias and a multiplicative "explored" mask
applied to the scores before softmax (masked scores are set to 0, so they
contribute exp(0)=1 to the softmax).

Sharding: 16 heads / 8 cores = 2 heads per core (tensor parallel over heads).
Each core computes projections for its 128 W-columns and full attention for
its 2 heads; the host concatenates the per-core [128, 2048] transposed
outputs. No collectives.
"""

import sys

for _p in ("/opt/trn_rl_repo",):
    if _p not in sys.path:
        sys.path.insert(0, _p)

import numpy as np
import ml_dtypes

BF16 = ml_dtypes.bfloat16

N = 2048
HID = 1024
HEADS = 16
DK = 64
NCORES = 8
HPC = HEADS // NCORES  # 2 heads per core
DC = HPC * DK  # 128 output columns per core
KT = HID // 128  # 8 contraction tiles
MT = N // 128  # 16 m tiles
NCH = N // 512  # 4 n chunks

_cache = {}


def _build():
    import concourse.bass as bass
    import concourse.bacc as bacc
    import concourse.mybir as mybir
    import concourse.tile as tile
    from concourse.masks import make_identity

    f32 = mybir.dt.float32
    bf16 = mybir.dt.bfloat16
    AF = mybir.ActivationFunctionType

    nc = bacc.Bacc("TRN2", target_bir_lowering=False, debug=False)

    xts = {t: nc.dram_tensor(f"xT{t}", [HID, N], bf16, kind="ExternalInput") for t in "qkv"}
    Ws = {t: nc.dram_tensor(f"W{t}", [HID, DC], bf16, kind="ExternalInput") for t in "qkv"}
    bs = {t: nc.dram_tensor(f"b{t}", [DC, 1], f32, kind="ExternalInput") for t in "qkv"}
    biasT = nc.dram_tensor("biasT", [HPC * N, N], bf16, kind="ExternalInput")
    invm = nc.dram_tensor("invmaskT", [N, N], mybir.dt.uint8, kind="ExternalInput")
    outT = nc.dram_tensor("outT", [DC, N], f32, kind="ExternalOutput")

    with tile.TileContext(nc) as tc:
        with (
            tc.tile_pool(name="constp", bufs=1) as constp,
            tc.tile_pool(name="xtp", bufs=2) as xtp,
            tc.tile_pool(name="pers", bufs=1) as pers,
            tc.tile_pool(name="maskp", bufs=1) as maskp,
            tc.tile_pool(name="biasp", bufs=6) as biasp,
            tc.tile_pool(name="ep", bufs=3) as ep,
            tc.tile_pool(name="normp", bufs=4) as normp,
            tc.tile_pool(name="outp", bufs=2) as outp,
            tc.tile_pool(name="ppsum", bufs=2, space="PSUM") as ppsum,
            tc.tile_pool(name="spsum", bufs=2, space="PSUM") as spsum,
            tc.tile_pool(name="opsum", bufs=2, space="PSUM") as opsum,
        ):
            ident = constp.tile([128, 128], bf16)
            make_identity(nc, ident)
            ones_bf = constp.tile([128, 512], bf16)
            nc.vector.memset(ones_bf, 1.0)
            ones_f32 = constp.tile([128, 64], f32)
            nc.vector.memset(ones_f32, 1.0)

            W_sb = {}
            b_sb = {}
            for t in "qkv":
                W_sb[t] = constp.tile([128, KT, DC], bf16, tag=f"w{t}", name=f"W{t}_sb")
                nc.sync.dma_start(
                    out=W_sb[t], in_=Ws[t].ap().rearrange("(kt p) m -> p kt m", p=128)
                )
                b_sb[t] = constp.tile([DC, 1], f32, tag=f"b{t}", name=f"b{t}_sb")
                nc.sync.dma_start(out=b_sb[t], in_=bs[t].ap())

            maskt = maskp.tile([128, MT, N], mybir.dt.uint8)
            nc.sync.dma_start(
                out=maskt, in_=invm.ap().rearrange("(mt p) n -> p mt n", p=128)
            )

            # ---- Phase 1: projections qhT/khT/vhT = W.T @ x.T (+ bias) ----
            proj = {}
            for t in "qkv":
                xt_sb = xtp.tile([128, KT, N], bf16, tag="xt")
                nc.sync.dma_start(
                    out=xt_sb, in_=xts[t].ap().rearrange("(kt p) n -> p kt n", p=128)
                )
                proj[t] = pers.tile([128, N], bf16, tag=f"proj{t}", name=f"proj{t}_sb")
                for ch in range(NCH):
                    ps = ppsum.tile([128, 512], f32, tag="pp")
                    for kt in range(KT):
                        nc.tensor.matmul(
                            ps,
                            lhsT=W_sb[t][:, kt, :],
                            rhs=xt_sb[:, kt, ch * 512 : (ch + 1) * 512],
                            start=(kt == 0),
                            stop=(kt == KT - 1),
                        )
                    nc.scalar.activation(
                        proj[t][:, ch * 512 : (ch + 1) * 512],
                        ps,
                        AF.Identity,
                        bias=b_sb[t],
                        scale=1.0,
                    )

            # vh' per head: [m-part, mt, 65] with col 64 = ones (softmax denom)
            vhp = [pers.tile([128, MT, DK + 1], bf16, tag=f"vhp{h}", name=f"vhp{h}_sb") for h in range(HPC)]
            for h in range(HPC):
                nc.vector.memset(vhp[h][:, :, DK : DK + 1], 1.0)
            for mb in range(MT):
                pstr = ppsum.tile([128, 128], bf16, tag="pp")
                nc.tensor.transpose(
                    pstr, proj["v"][:, mb * 128 : (mb + 1) * 128], ident
                )
                for h in range(HPC):
                    nc.scalar.activation(
                        vhp[h][:, mb, 0:DK], pstr[:, h * DK : (h + 1) * DK], AF.Copy
                    )

            # ---- Phase 2: attention ----
            for nch in range(NCH):
                n0 = nch * 512
                pouts = [opsum.tile([128, 512], f32, tag="po", name=f"pout{nch}_{h}") for h in range(HPC)]
                for mt in range(MT):
                    bts = []
                    for h in range(HPC):
                        bt = biasp.tile([128, 512], bf16, tag="bt")
                        nc.sync.dma_start(
                            out=bt,
                            in_=biasT.ap()[
                                h * N + mt * 128 : h * N + (mt + 1) * 128,
                                n0 : n0 + 512,
                            ],
                        )
                        bts.append(bt)
                    ps = spsum.tile([128, 1024], f32, tag="ps")
                    # scores^T: kh @ qh^T  (K=64; h0 rows 0-63, h1 rows 64-127)
                    for h in range(HPC):
                        nc.tensor.matmul(
                            ps[:, h * 512 : (h + 1) * 512],
                            lhsT=proj["k"][h * DK : (h + 1) * DK, mt * 128 : (mt + 1) * 128],
                            rhs=proj["q"][h * DK : (h + 1) * DK, n0 : n0 + 512],
                            start=True,
                            stop=False,
                        )
                    # += bias^T via identity matmul
                    for h in range(HPC):
                        nc.tensor.matmul(
                            ps[:, h * 512 : (h + 1) * 512],
                            lhsT=ident,
                            rhs=bts[h],
                            start=False,
                            stop=True,
                        )
                    et = ep.tile([128, 1024], bf16, tag="et")
                    nc.scalar.activation(et, ps, AF.Exp)
                    # masked entries -> exp(0) = 1
                    et3 = et.rearrange("p (h n) -> p h n", h=HPC)
                    for h in range(HPC):
                        nc.vector.copy_predicated(
                            et3[:, h, :], maskt[:, mt, n0 : n0 + 512], ones_bf
                        )
                    # out^T[d', n] += vh'.T @ e  (row 64 = Z)
                    for h in range(HPC):
                        nc.tensor.matmul(
                            pouts[h][0 : DK + 1, :],
                            lhsT=vhp[h][:, mt, :],
                            rhs=et3[:, h, :],
                            start=(mt == 0),
                            stop=(mt == MT - 1),
                        )
                # normalize: out = num * (1/Z), 1/Z = exp(-ln Z)
                for h in range(HPC):
                    nt = normp.tile([128, 512], f32, tag="nrm")
                    nc.scalar.activation(
                        nt[DK : DK + 1, :], pouts[h][DK : DK + 1, :], AF.Ln
                    )
                    rzs = normp.tile([128, 512], f32, tag="nrm")
                    nc.scalar.activation(
                        rzs[DK : DK + 1, :], nt[DK : DK + 1, :], AF.Exp, scale=-1.0
                    )
                    rzp = spsum.tile([128, 512], f32, tag="ps")
                    nc.tensor.matmul(
                        rzp[0:DK, :],
                        lhsT=ones_f32[DK : DK + 1, 0:DK],
                        rhs=rzs[DK : DK + 1, :],
                        start=True,
                        stop=True,
                    )
                    rzsb = normp.tile([128, 512], f32, tag="nrm")
                    nc.scalar.activation(rzsb[0:DK, :], rzp[0:DK, :], AF.Copy)
                    ot = outp.tile([128, 512], f32, tag="ot")
                    nc.vector.tensor_mul(ot[0:DK, :], pouts[h][0:DK, :], rzsb[0:DK, :])
                    nc.sync.dma_start(
                        out=outT.ap()[h * DK : (h + 1) * DK, n0 : n0 + 512],
                        in_=ot[0:DK, :],
                    )

    nc.compile()
    return nc


def stage_inputs(q, k, v, attn_bias, explored, Wq, bq, Wk, bk, Wv, bv):
    """Host-side sharding/layout staging. Returns in_maps for 8 cores."""
    scale = DK ** -0.5
    xT = {
        "q": np.ascontiguousarray(np.asarray(q, np.float32).T).astype(BF16),
        "k": np.ascontiguousarray(np.asarray(k, np.float32).T).astype(BF16),
        "v": np.ascontiguousarray(np.asarray(v, np.float32).T).astype(BF16),
    }
    Wq = np.asarray(Wq, np.float32) * scale
    bq = np.asarray(bq, np.float32) * scale
    Wk = np.asarray(Wk, np.float32)
    bk = np.asarray(bk, np.float32)
    Wv = np.asarray(Wv, np.float32)
    bv = np.asarray(bv, np.float32)
    attn_bias = np.asarray(attn_bias, np.float32)
    explored = np.asarray(explored)

    # inverted keep-mask, transposed: 1 where score must be zeroed
    invmask = np.zeros((N, N), dtype=np.uint8)
    invmask[1:, 1:] = (explored == 0).T.astype(np.uint8)

    in_maps = []
    for c in range(NCORES):
        cols = slice(c * DC, (c + 1) * DC)
        h0 = HPC * c
        bt = np.ascontiguousarray(
            attn_bias[h0 : h0 + HPC].transpose(0, 2, 1)
        ).astype(BF16).reshape(HPC * N, N)
        in_maps.append(
            {
                "xTq": xT["q"],
                "xTk": xT["k"],
                "xTv": xT["v"],
                "Wq": Wq[:, cols].astype(BF16),
                "Wk": Wk[:, cols].astype(BF16),
                "Wv": Wv[:, cols].astype(BF16),
                "bq": bq[cols].reshape(DC, 1).copy(),
                "bk": bk[cols].reshape(DC, 1).copy(),
                "bv": bv[cols].reshape(DC, 1).copy(),
                "biasT": bt,
                "invmaskT": invmask,
            }
        )
    return in_maps


def assemble_output(results):
    """results: list of 8 dicts with 'outT' [128, 2048] f32."""
    out = np.empty((N, HEADS * DK), dtype=np.float32)
    for c in range(NCORES):
        r = np.asarray(results[c]["outT"])
        for j in range(HPC):
            h = HPC * c + j
            out[:, h * DK : (h + 1) * DK] = r[j * DK : (j + 1) * DK, :].T
    return out


def get_compiled():
    if "nc" not in _cache:
        _cache["nc"] = _build()
    return _cache["nc"]


def kernel(**inputs) -> np.ndarray:
    from concourse.bass_utils import run_bass_kernel_spmd

    nc = get_compiled()
    in_maps = stage_inputs(**inputs)
    res = run_bass_kernel_spmd(nc, in_maps, core_ids=list(range(NCORES)))
    return assemble_output(res.results)
